# revision 43
# baseline (speedup 1.0000x reference)
"""MeshUnpool Trainium2 kernel (v5, 51.6us vs 78.8us baseline).

For every fine edge slot s in [0, 16384):
  - if s is a kept slot (s == keep_idx[j] for some j): out[s] = x_coarse[j]
  - else: out[s] = x_coarse[argmin_j |keep_idx[j] - s|]  (first-min tie-break)

v5 design (per core m; slots [2048m, 2048(m+1))):
  1. slot table over an 18-partition window (slot>>7 in [16m-1, 16m+17)):
     either host-packed (36KB upload; index-layout prep only) or built
     on-device by a windowed block-diagonal matmul scatter
     (DEVICE_SCATTER=True; ~19us of PE+DVE, measured slower).
  2. dual-f32-key prefix-max / suffix-min scans on [18, 128]; carry
     totals via tensor_reduce (parallel with the scans) + PE transpose
     + exclusive scans; distance/argmin + first-min tie-break decode
     on-device in i32.
  3. PE-transpose of the decoded src rows gives [128, 16] gather
     offsets; 16 single-column indirect DMAs (multi-column offset APs
     mis-execute on HW) gather bf16 x_coarse rows, DVE casts to f32,
     16 HWDGE writes on two queues.

Sharding: rows of the fine-edge dim, 2048 per core; window-relative
slot table per core; x_coarse uploaded as bf16 for the gather (rel
err ~3e-3, well under the 2e-2 gate).
"""

import os
import sys

import numpy as np

E_FINE = 16384
E_COARSE = 8192
C = 512
N_CORES = 8
SLICE = E_FINE // N_CORES  # 2048
P = 128
KC = E_COARSE // P  # 64 keep chunks (j = 128*c + jp)
WP = 18  # window partitions (slot>>7 in [16m-1, 16m+17))
G = 2  # chunks per matmul group
KCP = 66  # chunks padded to a multiple of G (2 dummy zero chunks)
NGRP = KCP // G  # 33
PW = 64  # lhsT cols per chunk: u at [0:32), v at [32:64) (32-aligned blocks)
OHW = 32  # one-hot width for the window compare
SLAB = 12  # chunks per one-hot slab (multiple of G)
NSLAB = (KCP + SLAB - 1) // SLAB  # 6

POS_OFF = 16384  # keeps scan keys strictly positive
R_SENT = 8388608.0  # 2^23 sentinel for the suffix-min scans

DEVICE_SCATTER = False  # False: host builds the (u, v) slot table
WRITE_BF16 = False  # True: y stored bf16, upcast on host (measured slower)

_NC_CACHE = {}


def _ensure_paths():
    for p in ("/opt/trn_rl_repo", "/root/.axon_site/_ro/trn_rl_repo"):
        if os.path.isdir(p) and p not in sys.path:
            sys.path.append(p)


def build_program(nc, bass, mybir, tile):
    f32 = mybir.dt.float32
    i32 = mybir.dt.int32
    bf16 = mybir.dt.bfloat16
    Alu = mybir.AluOpType

    xch = nc.dram_tensor("xch", [E_COARSE, C], bf16, kind="ExternalInput")
    # iota ramp 0..127 repeated in every partition
    bpk = nc.dram_tensor("bpk", [P, P], bf16, kind="ExternalInput")
    if DEVICE_SCATTER:
        # hbw[jp, c] = (keep_idx[128c+jp] >> 7) - (16m - 1); lbw = keep & 127
        hbw = nc.dram_tensor("hbw", [P, KCP], bf16, kind="ExternalInput")
        lbw = nc.dram_tensor("lbw", [P, KCP], bf16, kind="ExternalInput")
        # crpb[jp, c, w] = c  (payload-v ramp, broadcast along w)
        crpb = nc.dram_tensor("crpb", [P, KCP, OHW], bf16, kind="ExternalInput")
        # jpk[jp, 0] = jp + 1
        jpk = nc.dram_tensor("jpk", [P, 1], f32, kind="ExternalInput")
    else:
        # host-built slot table: u cols [0:P), v cols [P:2P)
        tbl = nc.dram_tensor("tbl", [WP, 2 * P], f32, kind="ExternalInput")
    # fpk: p128m1 | p64 | ident18 (f32 per-core ramps)
    fpk = nc.dram_tensor("fpk", [WP, 2 * P + WP], f32, kind="ExternalInput")
    # ipk: pos' | 2*pos' (int32)
    ipk = nc.dram_tensor("ipk", [WP, 2 * P], i32, kind="ExternalInput")
    ydt = bf16 if WRITE_BF16 else f32
    y = nc.dram_tensor("y", [SLICE, C], ydt, kind="ExternalOutput")

    with tile.TileContext(nc) as tc:
        with (
            tc.tile_pool(name="sb", bufs=1) as sb,
            tc.tile_pool(name="ps", bufs=1, space="PSUM") as ps,
        ):
            fpk_t = sb.tile([WP, 2 * P + WP], f32)
            nc.sync.dma_start(fpk_t[:], fpk[:])
            ipk_t = sb.tile([WP, 2 * P], i32)
            nc.sync.dma_start(ipk_t[:], ipk[:])
            p128m1 = fpk_t[:, 0:P]
            p64 = fpk_t[:, P : 2 * P]
            ident = fpk_t[:, 2 * P : 2 * P + WP]
            pos_i = ipk_t[:, 0:P]
            pos2_i = ipk_t[:, P : 2 * P]

            if DEVICE_SCATTER:
                bpk_t = sb.tile([P, P], bf16)
                nc.sync.dma_start(bpk_t[:], bpk[:])
                hbw_t = sb.tile([P, KCP], bf16)
                nc.scalar.dma_start(hbw_t[:], hbw[:])
                lbw_t = sb.tile([P, KCP], bf16)
                nc.scalar.dma_start(lbw_t[:], lbw[:])
                crpb_t = sb.tile([P, KCP, OHW], bf16)
                nc.scalar.dma_start(crpb_t[:], crpb[:])
                jpk_t = sb.tile([P, 1], f32)
                nc.sync.dma_start(jpk_t[:], jpk[:])
                iota = bpk_t[:, 0:P]
                jp1 = jpk_t[:, 0:1]

                # ---- one-hot slabs + payload slabs (stride-0 broadcasts)
                lb_sl = []
                pay_sl = []
                for s in range(NSLAB):
                    n = min(SLAB, KCP - SLAB * s)
                    cs = slice(SLAB * s, SLAB * s + n)
                    lbt = sb.tile([P, n, P], bf16, tag=f"lb{s}")
                    eng = nc.vector
                    eng.tensor_tensor(
                        lbt[:],
                        iota.unsqueeze(1).to_broadcast([P, n, P]),
                        lbw_t[:, cs].unsqueeze(2).to_broadcast([P, n, P]),
                        Alu.is_equal,
                    )
                    lb_sl.append(lbt)
                    # pay[:, :, 0:OHW] = (jp+1)*onehot ; [:, OHW:PW] = c*onehot
                    # (one-hot width 32 > WP=18; cols 18..31 never match)
                    oh = sb.tile([P, n, OHW], bf16, tag=f"oh{s}")
                    nc.vector.tensor_tensor(
                        oh[:],
                        bpk_t[:, 0:OHW].unsqueeze(1).to_broadcast([P, n, OHW]),
                        hbw_t[:, cs].unsqueeze(2).to_broadcast([P, n, OHW]),
                        Alu.is_equal,
                    )
                    pay = sb.tile([P, n, PW], bf16, tag=f"pay{s}")
                    nc.vector.tensor_scalar(
                        pay[:, :, 0:OHW], oh[:], jp1, None, Alu.mult
                    )
                    nc.vector.tensor_tensor(
                        pay[:, :, OHW:PW], oh[:], crpb_t[:, cs, :], Alu.mult
                    )
                    pay_sl.append(pay)

                # ---- block-diagonal grouped matmul scatter
                tps = ps.tile([PW * G, P * G], f32)
                for g in range(NGRP):
                    c0 = G * g
                    s = c0 // SLAB
                    lo = c0 - SLAB * s
                    lhsT = pay_sl[s][:, lo : lo + G, :]
                    rhs = lb_sl[s][:, lo : lo + G, :]
                    nc.tensor.matmul(
                        tps[:],
                        lhsT,
                        rhs,
                        start=(g == 0),
                        stop=(g == NGRP - 1),
                    )

                # ---- combine diagonal blocks -> table_u, table_v [WP, P]
                # (DVE may read at most one PSUM operand per instruction)
                tuc = sb.tile([WP, P], f32)
                nc.vector.tensor_copy(tuc[:], tps[0:WP, 0:P])
                tu = sb.tile([WP, P], f32)
                nc.vector.tensor_tensor(
                    tu[:], tuc[:], tps[PW : PW + WP, P : 2 * P], Alu.add
                )
                tvc = sb.tile([WP, P], f32)
                nc.vector.tensor_copy(tvc[:], tps[OHW : OHW + WP, 0:P])
                tv = sb.tile([WP, P], f32)
                nc.vector.tensor_tensor(
                    tv[:], tvc[:], tps[PW + OHW : PW + OHW + WP, P : 2 * P], Alu.add
                )
            else:
                tbl_t = sb.tile([WP, 2 * P], f32)
                nc.sync.dma_start(tbl_t[:], tbl[:])
                tu = tbl_t[:, 0:P]
                tv = tbl_t[:, P : 2 * P]

            # ---- keys: key1 = kept*(128*pos' + jp), key2 = kept*(64*pos' + c)
            mk = sb.tile([WP, P], f32)
            nc.vector.tensor_scalar(mk[:], tu[:], 0.0, None, Alu.is_gt)
            k1r = sb.tile([WP, P], f32)
            nc.vector.tensor_tensor(k1r[:], tu[:], p128m1, Alu.add)
            key1 = sb.tile([WP, P], f32)
            nc.vector.tensor_tensor(key1[:], k1r[:], mk[:], Alu.mult)
            k2r = sb.tile([WP, P], f32)
            nc.vector.tensor_tensor(k2r[:], tv[:], p64, Alu.add)
            key2 = sb.tile([WP, P], f32)
            nc.vector.tensor_tensor(key2[:], k2r[:], mk[:], Alu.mult)
            msk = sb.tile([WP, P], f32)
            nc.vector.tensor_scalar(msk[:], key1[:], 0.0, None, Alu.is_equal)
            r1 = sb.tile([WP, P], f32)
            nc.vector.scalar_tensor_tensor(
                r1[:], msk[:], R_SENT, key1[:], Alu.mult, Alu.add
            )
            r2 = sb.tile([WP, P], f32)
            nc.vector.scalar_tensor_tensor(
                r2[:], msk[:], R_SENT, key2[:], Alu.mult, Alu.add
            )

            # ---- per-partition scans; l1s|l2s|r1s|r2s packed in one tile
            sc = sb.tile([WP, 4 * P], f32)
            nc.vector.tensor_tensor_scan(
                sc[:, 0:P], key1[:], key1[:], 0.0, Alu.max, Alu.max
            )
            nc.vector.tensor_tensor_scan(
                sc[:, P : 2 * P], key2[:], key2[:], 0.0, Alu.max, Alu.max
            )
            nc.vector.tensor_tensor_scan(
                sc[:, 3 * P - 1 : 2 * P - 1 : -1],
                r1[:, P - 1 :: -1],
                r1[:, P - 1 :: -1],
                R_SENT,
                Alu.min,
                Alu.min,
            )
            nc.vector.tensor_tensor_scan(
                sc[:, 4 * P - 1 : 3 * P - 1 : -1],
                r2[:, P - 1 :: -1],
                r2[:, P - 1 :: -1],
                R_SENT,
                Alu.min,
                Alu.min,
            )

            # ---- cross-partition carries: totals via reduce (parallel with
            # the scans), transpose, exclusive scan, transpose back
            totL = sb.tile([WP, 2], f32)
            nc.vector.tensor_reduce(
                totL[:, 0:1], key1[:], mybir.AxisListType.X, Alu.max
            )
            nc.vector.tensor_reduce(
                totL[:, 1:2], key2[:], mybir.AxisListType.X, Alu.max
            )
            totR = sb.tile([WP, 2], f32)
            nc.vector.tensor_reduce(
                totR[:, 0:1], r1[:], mybir.AxisListType.X, Alu.min
            )
            nc.vector.tensor_reduce(
                totR[:, 1:2], r2[:], mybir.AxisListType.X, Alu.min
            )
            totL_ps = ps.tile([2, WP], f32)
            nc.tensor.transpose(totL_ps[:], totL[:], ident)
            totR_ps = ps.tile([2, WP], f32)
            nc.tensor.transpose(totR_ps[:], totR[:], ident)
            totL_T = sb.tile([2, WP], f32)
            nc.vector.tensor_copy(totL_T[:], totL_ps[:])
            totR_T = sb.tile([2, WP], f32)
            nc.vector.tensor_copy(totR_T[:], totR_ps[:])
            exL = sb.tile([2, WP], f32)
            nc.vector.memset(exL[:, 0:1], 0.0)
            nc.vector.tensor_tensor_scan(
                exL[:, 1:WP],
                totL_T[:, 0 : WP - 1],
                totL_T[:, 0 : WP - 1],
                0.0,
                Alu.max,
                Alu.max,
            )
            exR = sb.tile([2, WP], f32)
            nc.vector.memset(exR[:, WP - 1 : WP], R_SENT)
            nc.vector.tensor_tensor_scan(
                exR[:, WP - 2 :: -1],
                totR_T[:, WP - 1 : 0 : -1],
                totR_T[:, WP - 1 : 0 : -1],
                R_SENT,
                Alu.min,
                Alu.min,
            )
            carL_ps = ps.tile([WP, 2], f32)
            nc.tensor.transpose(carL_ps[:], exL[:], ident[0:2, 0:2])
            carR_ps = ps.tile([WP, 2], f32)
            nc.tensor.transpose(carR_ps[:], exR[:], ident[0:2, 0:2])
            carL = sb.tile([WP, 2], f32)
            nc.vector.tensor_copy(carL[:], carL_ps[:])
            carR = sb.tile([WP, 2], f32)
            nc.vector.tensor_copy(carR[:], carR_ps[:])

            # ---- combine carries, then integer decode
            nc.vector.tensor_scalar_max(sc[:, 0:P], sc[:, 0:P], carL[:, 0:1])
            nc.vector.tensor_scalar_max(
                sc[:, P : 2 * P], sc[:, P : 2 * P], carL[:, 1:2]
            )
            nc.vector.tensor_scalar_min(
                sc[:, 2 * P : 3 * P], sc[:, 2 * P : 3 * P], carR[:, 0:1]
            )
            nc.vector.tensor_scalar_min(
                sc[:, 3 * P : 4 * P], sc[:, 3 * P : 4 * P], carR[:, 1:2]
            )
            l1m = sb.tile([WP, P], i32)
            nc.vector.tensor_copy(l1m[:], sc[:, 0:P])
            l2m = sb.tile([WP, P], i32)
            nc.vector.tensor_copy(l2m[:], sc[:, P : 2 * P])
            r1m = sb.tile([WP, P], i32)
            nc.vector.tensor_copy(r1m[:], sc[:, 2 * P : 3 * P])
            r2m = sb.tile([WP, P], i32)
            nc.vector.tensor_copy(r2m[:], sc[:, 3 * P : 4 * P])

            slot_l = sb.tile([WP, P], i32)
            nc.vector.tensor_scalar(
                slot_l[:], l1m[:], 7, None, Alu.arith_shift_right
            )
            slot_r = sb.tile([WP, P], i32)
            nc.vector.tensor_scalar(
                slot_r[:], r1m[:], 7, None, Alu.arith_shift_right
            )
            # sd = slot_l + slot_r - 2*pos'  (>0: left closer; <0: right)
            ssum = sb.tile([WP, P], i32)
            nc.vector.tensor_tensor(ssum[:], slot_l[:], slot_r[:], Alu.add)
            sd = sb.tile([WP, P], i32)
            nc.vector.tensor_tensor(sd[:], ssum[:], pos2_i, Alu.subtract)
            # j = 128*c + jp  (u = jp+1 payload -> key stores jp; v = c)
            jcl = sb.tile([WP, P], i32)
            nc.vector.tensor_scalar(
                jcl[:], l2m[:], 63, 7, Alu.bitwise_and, Alu.arith_shift_left
            )
            jpl = sb.tile([WP, P], i32)
            nc.vector.tensor_scalar(jpl[:], l1m[:], 127, None, Alu.bitwise_and)
            jl = sb.tile([WP, P], i32)
            nc.vector.tensor_tensor(jl[:], jcl[:], jpl[:], Alu.bitwise_or)
            jcr = sb.tile([WP, P], i32)
            nc.vector.tensor_scalar(
                jcr[:], r2m[:], 63, 7, Alu.bitwise_and, Alu.arith_shift_left
            )
            jpr = sb.tile([WP, P], i32)
            nc.vector.tensor_scalar(jpr[:], r1m[:], 127, None, Alu.bitwise_and)
            jr = sb.tile([WP, P], i32)
            nc.vector.tensor_tensor(jr[:], jcr[:], jpr[:], Alu.bitwise_or)

            ml = sb.tile([WP, P], i32)
            nc.vector.tensor_scalar(ml[:], sd[:], 0, None, Alu.is_gt)
            mr = sb.tile([WP, P], i32)
            nc.vector.tensor_scalar(mr[:], sd[:], 0, None, Alu.is_lt)
            src = sb.tile([WP, P], i32)
            nc.vector.tensor_tensor(src[:], jl[:], jr[:], Alu.min)
            nc.vector.copy_predicated(src[:], mr[:], jr[:])
            nc.vector.copy_predicated(src[:], ml[:], jl[:])
            srcf = sb.tile([WP, P], f32)
            nc.vector.tensor_copy(srcf[:], src[:])

            # ---- transpose src rows -> [128, WP]; own slice = cols 1..17
            g_ps = ps.tile([P, WP], f32)
            nc.tensor.transpose(g_ps[:], srcf[:], ident)
            g_i = sb.tile([P, 16], i32)
            nc.vector.tensor_copy(g_i[:], g_ps[:, 1 : WP - 1])

            # ---- gather + cast + write, one 128-row block per indirect DMA
            # (multi-column offset APs mis-execute on HW; single-column is
            # proven.  Q7 descriptor generation pipelines with SDMA + writes.)
            yview = y[:].rearrange("(r p) c -> p r c", p=P)
            for k in range(16):
                gt = sb.tile([P, C], bf16, tag=f"gt{k % 8}")
                nc.gpsimd.indirect_dma_start(
                    out=gt[:],
                    out_offset=None,
                    in_=xch[:],
                    in_offset=bass.IndirectOffsetOnAxis(
                        ap=g_i[:, k : k + 1], axis=0
                    ),
                )
                if WRITE_BF16:
                    wsrc = gt[:]
                else:
                    gf = sb.tile([P, C], f32, tag=f"gf{k % 8}")
                    nc.vector.tensor_copy(gf[:], gt[:])
                    wsrc = gf[:]
                weng = nc.sync if k % 2 == 0 else nc.scalar
                weng.dma_start(yview[:, k : k + 1, :], wsrc.unsqueeze(1))

    return {"y": y}


def host_inputs(x_coarse, keep_idx):
    import ml_dtypes

    bf = ml_dtypes.bfloat16
    x32 = np.ascontiguousarray(np.asarray(x_coarse), dtype=np.float32)
    ki = np.ascontiguousarray(np.asarray(keep_idx), dtype=np.int64).reshape(-1)
    assert ki.shape == (E_COARSE,)

    # window-coverage guard: every slot's nearest kept neighbor (each side,
    # where one exists) must lie within the per-core window margin (128).
    kept_sorted = np.sort(ki)
    gaps = np.diff(kept_sorted)
    assert gaps.max() <= P, f"kept gap {gaps.max()} exceeds window margin"
    assert kept_sorted[0] <= P and (E_FINE - 1 - kept_sorted[-1]) <= P

    keep_w = ki.reshape(KC, P).T  # [jp, c] = keep_idx[128c+jp]
    hb = np.full((P, KCP), -1000, dtype=np.int64)
    lb = np.full((P, KCP), -1000, dtype=np.int64)
    hb[:, :KC] = keep_w >> 7
    lb[:, :KC] = keep_w & 127

    t = np.arange(P)
    iota_a = np.tile(t[None, :], (P, 1)).astype(bf)
    crpb_a = np.tile(
        np.minimum(np.arange(KCP, dtype=np.float32), KC - 1)[None, :, None],
        (P, 1, OHW),
    ).astype(bf)
    jpk_a = (t[:, None] + 1).astype(np.float32)
    xch_a = x32.astype(bf)

    base = {
        "xch": xch_a,
        "bpk": np.ascontiguousarray(iota_a),
    }
    if DEVICE_SCATTER:
        base["crpb"] = np.ascontiguousarray(crpb_a)
        base["jpk"] = np.ascontiguousarray(jpk_a)
        base["lbw"] = np.ascontiguousarray(lb.astype(bf))

    in_maps = []
    for m in range(N_CORES):
        wbase = 16 * m - 1
        w = np.arange(WP)
        posp = POS_OFF + P * (wbase + w[:, None]) + t[None, :]
        p128m1 = (128.0 * posp - 1).astype(np.float32)
        p64 = (64.0 * posp).astype(np.float32)
        ident_a = np.eye(WP, dtype=np.float32)
        fpk_a = np.concatenate([p128m1, p64, ident_a], axis=1)
        ipk_a = np.concatenate([posp, 2 * posp], axis=1).astype(np.int32)
        im = dict(
            base,
            fpk=np.ascontiguousarray(fpk_a),
            ipk=np.ascontiguousarray(ipk_a),
        )
        if DEVICE_SCATTER:
            im["hbw"] = np.ascontiguousarray((hb - wbase).astype(bf))
        else:
            tbl_a = np.zeros((WP, 2 * P), dtype=np.float32)
            winm = (hb >= wbase) & (hb < wbase + WP)
            jj, cc = np.nonzero(winm)
            tbl_a[hb[jj, cc] - wbase, lb[jj, cc]] = jj + 1.0
            tbl_a[hb[jj, cc] - wbase, P + lb[jj, cc]] = cc
            im["tbl"] = np.ascontiguousarray(tbl_a)
        in_maps.append(im)
    return in_maps


def assemble_output(res):
    out = np.concatenate(
        [np.asarray(res.results[m]["y"]) for m in range(N_CORES)], axis=0
    )
    return np.ascontiguousarray(out.astype(np.float32))


def _get_nc():
    if "nc" in _NC_CACHE:
        return _NC_CACHE["nc"]
    _ensure_paths()
    from concourse import bass, mybir
    import concourse.bacc as bacc
    import concourse.tile as tile

    nc = bacc.Bacc(
        "TRN2", target_bir_lowering=False, debug=False, dynamic_dma_scratch_size=16384
    )
    build_program(nc, bass, mybir, tile)
    nc.compile()
    _NC_CACHE["nc"] = nc
    return nc


def run_on_hw(in_maps, trace=False, **kwargs):
    _ensure_paths()
    from concourse.bass_utils import run_bass_kernel_spmd

    nc = _get_nc()
    return run_bass_kernel_spmd(
        nc, in_maps, core_ids=list(range(N_CORES)), trace=trace, **kwargs
    )


def kernel(x_coarse, keep_idx, E_fine=None, **_unused):
    in_maps = host_inputs(x_coarse, keep_idx)
    res = run_on_hw(in_maps)
    return assemble_output(res)


# revision 47
# speedup vs baseline: 1.1395x; 1.1395x over previous
"""MeshUnpool Trainium2 kernel (v5, 51.6us vs 78.8us baseline).

For every fine edge slot s in [0, 16384):
  - if s is a kept slot (s == keep_idx[j] for some j): out[s] = x_coarse[j]
  - else: out[s] = x_coarse[argmin_j |keep_idx[j] - s|]  (first-min tie-break)

v5 design (per core m; slots [2048m, 2048(m+1))):
  1. slot table over an 18-partition window (slot>>7 in [16m-1, 16m+17)):
     either host-packed (36KB upload; index-layout prep only) or built
     on-device by a windowed block-diagonal matmul scatter
     (DEVICE_SCATTER=True; ~19us of PE+DVE, measured slower).
  2. dual-f32-key prefix-max / suffix-min scans on [18, 128]; carry
     totals via tensor_reduce (parallel with the scans) + PE transpose
     + exclusive scans; distance/argmin + first-min tie-break decode
     on-device in i32.
  3. PE-transpose of the decoded src rows gives [128, 16] gather
     offsets; 16 single-column indirect DMAs (multi-column offset APs
     mis-execute on HW) gather bf16 x_coarse rows, DVE casts to f32,
     16 HWDGE writes on two queues.

Sharding: rows of the fine-edge dim, 2048 per core; window-relative
slot table per core; x_coarse uploaded as bf16 for the gather (rel
err ~3e-3, well under the 2e-2 gate).
"""

import os
import sys

import numpy as np

E_FINE = 16384
E_COARSE = 8192
C = 512
N_CORES = 8
SLICE = E_FINE // N_CORES  # 2048
P = 128
KC = E_COARSE // P  # 64 keep chunks (j = 128*c + jp)
WP = 18  # window partitions (slot>>7 in [16m-1, 16m+17))
G = 2  # chunks per matmul group
KCP = 66  # chunks padded to a multiple of G (2 dummy zero chunks)
NGRP = KCP // G  # 33
PW = 64  # lhsT cols per chunk: u at [0:32), v at [32:64) (32-aligned blocks)
OHW = 32  # one-hot width for the window compare
SLAB = 12  # chunks per one-hot slab (multiple of G)
NSLAB = (KCP + SLAB - 1) // SLAB  # 6

POS_OFF = 16384  # keeps scan keys strictly positive
R_SENT = 8388608.0  # 2^23 sentinel for the suffix-min scans

DEVICE_SCATTER = False  # False: host builds the (u, v) slot table
WRITE_BF16 = False  # True: y stored bf16, upcast on host (measured slower)

_NC_CACHE = {}


def _ensure_paths():
    for p in ("/opt/trn_rl_repo", "/root/.axon_site/_ro/trn_rl_repo"):
        if os.path.isdir(p) and p not in sys.path:
            sys.path.append(p)


def build_program(nc, bass, mybir, tile):
    f32 = mybir.dt.float32
    i32 = mybir.dt.int32
    bf16 = mybir.dt.bfloat16
    Alu = mybir.AluOpType

    xch = nc.dram_tensor("xch", [E_COARSE, C], bf16, kind="ExternalInput")
    # iota ramp 0..127 repeated in every partition
    bpk = nc.dram_tensor("bpk", [P, P], bf16, kind="ExternalInput")
    if DEVICE_SCATTER:
        # hbw[jp, c] = (keep_idx[128c+jp] >> 7) - (16m - 1); lbw = keep & 127
        hbw = nc.dram_tensor("hbw", [P, KCP], bf16, kind="ExternalInput")
        lbw = nc.dram_tensor("lbw", [P, KCP], bf16, kind="ExternalInput")
        # crpb[jp, c, w] = c  (payload-v ramp, broadcast along w)
        crpb = nc.dram_tensor("crpb", [P, KCP, OHW], bf16, kind="ExternalInput")
        # jpk[jp, 0] = jp + 1
        jpk = nc.dram_tensor("jpk", [P, 1], f32, kind="ExternalInput")
    else:
        # host-built slot table: u cols [0:P), v cols [P:2P)
        tbl = nc.dram_tensor("tbl", [WP, 2 * P], f32, kind="ExternalInput")
    # fpk: p128m1 | p64 | ident18 (f32 per-core ramps)
    fpk = nc.dram_tensor("fpk", [WP, 2 * P + WP], f32, kind="ExternalInput")
    # ipk: pos' | 2*pos' (int32)
    ipk = nc.dram_tensor("ipk", [WP, 2 * P], i32, kind="ExternalInput")
    ydt = bf16 if WRITE_BF16 else f32
    y = nc.dram_tensor("y", [SLICE, C], ydt, kind="ExternalOutput")

    with tile.TileContext(nc) as tc:
        with (
            tc.tile_pool(name="sb", bufs=1) as sb,
            tc.tile_pool(name="ps", bufs=1, space="PSUM") as ps,
        ):
            fpk_t = sb.tile([WP, 2 * P + WP], f32)
            nc.sync.dma_start(fpk_t[:], fpk[:])
            ipk_t = sb.tile([WP, 2 * P], i32)
            nc.sync.dma_start(ipk_t[:], ipk[:])
            p128m1 = fpk_t[:, 0:P]
            p64 = fpk_t[:, P : 2 * P]
            ident = fpk_t[:, 2 * P : 2 * P + WP]
            pos_i = ipk_t[:, 0:P]
            pos2_i = ipk_t[:, P : 2 * P]

            if DEVICE_SCATTER:
                bpk_t = sb.tile([P, P], bf16)
                nc.sync.dma_start(bpk_t[:], bpk[:])
                hbw_t = sb.tile([P, KCP], bf16)
                nc.scalar.dma_start(hbw_t[:], hbw[:])
                lbw_t = sb.tile([P, KCP], bf16)
                nc.scalar.dma_start(lbw_t[:], lbw[:])
                crpb_t = sb.tile([P, KCP, OHW], bf16)
                nc.scalar.dma_start(crpb_t[:], crpb[:])
                jpk_t = sb.tile([P, 1], f32)
                nc.sync.dma_start(jpk_t[:], jpk[:])
                iota = bpk_t[:, 0:P]
                jp1 = jpk_t[:, 0:1]

                # ---- one-hot slabs + payload slabs (stride-0 broadcasts)
                lb_sl = []
                pay_sl = []
                for s in range(NSLAB):
                    n = min(SLAB, KCP - SLAB * s)
                    cs = slice(SLAB * s, SLAB * s + n)
                    lbt = sb.tile([P, n, P], bf16, tag=f"lb{s}")
                    eng = nc.vector
                    eng.tensor_tensor(
                        lbt[:],
                        iota.unsqueeze(1).to_broadcast([P, n, P]),
                        lbw_t[:, cs].unsqueeze(2).to_broadcast([P, n, P]),
                        Alu.is_equal,
                    )
                    lb_sl.append(lbt)
                    # pay[:, :, 0:OHW] = (jp+1)*onehot ; [:, OHW:PW] = c*onehot
                    # (one-hot width 32 > WP=18; cols 18..31 never match)
                    oh = sb.tile([P, n, OHW], bf16, tag=f"oh{s}")
                    nc.vector.tensor_tensor(
                        oh[:],
                        bpk_t[:, 0:OHW].unsqueeze(1).to_broadcast([P, n, OHW]),
                        hbw_t[:, cs].unsqueeze(2).to_broadcast([P, n, OHW]),
                        Alu.is_equal,
                    )
                    pay = sb.tile([P, n, PW], bf16, tag=f"pay{s}")
                    nc.vector.tensor_scalar(
                        pay[:, :, 0:OHW], oh[:], jp1, None, Alu.mult
                    )
                    nc.vector.tensor_tensor(
                        pay[:, :, OHW:PW], oh[:], crpb_t[:, cs, :], Alu.mult
                    )
                    pay_sl.append(pay)

                # ---- block-diagonal grouped matmul scatter
                tps = ps.tile([PW * G, P * G], f32)
                for g in range(NGRP):
                    c0 = G * g
                    s = c0 // SLAB
                    lo = c0 - SLAB * s
                    lhsT = pay_sl[s][:, lo : lo + G, :]
                    rhs = lb_sl[s][:, lo : lo + G, :]
                    nc.tensor.matmul(
                        tps[:],
                        lhsT,
                        rhs,
                        start=(g == 0),
                        stop=(g == NGRP - 1),
                    )

                # ---- combine diagonal blocks -> table_u, table_v [WP, P]
                # (DVE may read at most one PSUM operand per instruction)
                tuc = sb.tile([WP, P], f32)
                nc.vector.tensor_copy(tuc[:], tps[0:WP, 0:P])
                tu = sb.tile([WP, P], f32)
                nc.vector.tensor_tensor(
                    tu[:], tuc[:], tps[PW : PW + WP, P : 2 * P], Alu.add
                )
                tvc = sb.tile([WP, P], f32)
                nc.vector.tensor_copy(tvc[:], tps[OHW : OHW + WP, 0:P])
                tv = sb.tile([WP, P], f32)
                nc.vector.tensor_tensor(
                    tv[:], tvc[:], tps[PW + OHW : PW + OHW + WP, P : 2 * P], Alu.add
                )
            else:
                tbl_t = sb.tile([WP, 2 * P], f32)
                nc.sync.dma_start(tbl_t[:], tbl[:])
                tu = tbl_t[:, 0:P]
                tv = tbl_t[:, P : 2 * P]

            # ---- keys: key1 = kept*(128*pos' + jp), key2 = kept*(64*pos' + c)
            if DEVICE_SCATTER:
                mk = sb.tile([WP, P], f32)
                nc.vector.tensor_scalar(mk[:], tu[:], 0.0, None, Alu.is_gt)
                k1r = sb.tile([WP, P], f32)
                nc.vector.tensor_tensor(k1r[:], tu[:], p128m1, Alu.add)
                key1 = sb.tile([WP, P], f32)
                nc.vector.tensor_tensor(key1[:], k1r[:], mk[:], Alu.mult)
                k2r = sb.tile([WP, P], f32)
                nc.vector.tensor_tensor(k2r[:], tv[:], p64, Alu.add)
                key2 = sb.tile([WP, P], f32)
                nc.vector.tensor_tensor(key2[:], k2r[:], mk[:], Alu.mult)
            else:
                # host table already stores the biased keys (0 = empty)
                key1 = tu
                key2 = tv
            msk = sb.tile([WP, P], f32)
            nc.vector.tensor_scalar(msk[:], key1[:], 0.0, None, Alu.is_equal)
            r1 = sb.tile([WP, P], f32)
            nc.vector.scalar_tensor_tensor(
                r1[:], msk[:], R_SENT, key1[:], Alu.mult, Alu.add
            )
            r2 = sb.tile([WP, P], f32)
            nc.vector.scalar_tensor_tensor(
                r2[:], msk[:], R_SENT, key2[:], Alu.mult, Alu.add
            )

            # ---- per-partition scans; l1s|l2s|r1s|r2s packed in one tile
            sc = sb.tile([WP, 4 * P], f32)
            nc.vector.tensor_tensor_scan(
                sc[:, 0:P], key1[:], key1[:], 0.0, Alu.max, Alu.max
            )
            nc.vector.tensor_tensor_scan(
                sc[:, P : 2 * P], key2[:], key2[:], 0.0, Alu.max, Alu.max
            )
            nc.vector.tensor_tensor_scan(
                sc[:, 3 * P - 1 : 2 * P - 1 : -1],
                r1[:, P - 1 :: -1],
                r1[:, P - 1 :: -1],
                R_SENT,
                Alu.min,
                Alu.min,
            )
            nc.vector.tensor_tensor_scan(
                sc[:, 4 * P - 1 : 3 * P - 1 : -1],
                r2[:, P - 1 :: -1],
                r2[:, P - 1 :: -1],
                R_SENT,
                Alu.min,
                Alu.min,
            )

            # ---- cross-partition carries: totals via reduce (parallel with
            # the scans), transpose, exclusive scan, transpose back
            totL = sb.tile([WP, 2], f32)
            nc.vector.tensor_reduce(
                totL[:, 0:1], key1[:], mybir.AxisListType.X, Alu.max
            )
            nc.vector.tensor_reduce(
                totL[:, 1:2], key2[:], mybir.AxisListType.X, Alu.max
            )
            totR = sb.tile([WP, 2], f32)
            nc.vector.tensor_reduce(
                totR[:, 0:1], r1[:], mybir.AxisListType.X, Alu.min
            )
            nc.vector.tensor_reduce(
                totR[:, 1:2], r2[:], mybir.AxisListType.X, Alu.min
            )
            totL_ps = ps.tile([2, WP], f32)
            nc.tensor.transpose(totL_ps[:], totL[:], ident)
            totR_ps = ps.tile([2, WP], f32)
            nc.tensor.transpose(totR_ps[:], totR[:], ident)
            totL_T = sb.tile([2, WP], f32)
            nc.vector.tensor_copy(totL_T[:], totL_ps[:])
            totR_T = sb.tile([2, WP], f32)
            nc.vector.tensor_copy(totR_T[:], totR_ps[:])
            exL = sb.tile([2, WP], f32)
            nc.vector.memset(exL[:, 0:1], 0.0)
            nc.vector.tensor_tensor_scan(
                exL[:, 1:WP],
                totL_T[:, 0 : WP - 1],
                totL_T[:, 0 : WP - 1],
                0.0,
                Alu.max,
                Alu.max,
            )
            exR = sb.tile([2, WP], f32)
            nc.vector.memset(exR[:, WP - 1 : WP], R_SENT)
            nc.vector.tensor_tensor_scan(
                exR[:, WP - 2 :: -1],
                totR_T[:, WP - 1 : 0 : -1],
                totR_T[:, WP - 1 : 0 : -1],
                R_SENT,
                Alu.min,
                Alu.min,
            )
            carL_ps = ps.tile([WP, 2], f32)
            nc.tensor.transpose(carL_ps[:], exL[:], ident[0:2, 0:2])
            carR_ps = ps.tile([WP, 2], f32)
            nc.tensor.transpose(carR_ps[:], exR[:], ident[0:2, 0:2])
            carL = sb.tile([WP, 2], f32)
            nc.vector.tensor_copy(carL[:], carL_ps[:])
            carR = sb.tile([WP, 2], f32)
            nc.vector.tensor_copy(carR[:], carR_ps[:])

            # ---- combine carries, then integer decode
            nc.vector.tensor_scalar_max(sc[:, 0:P], sc[:, 0:P], carL[:, 0:1])
            nc.vector.tensor_scalar_max(
                sc[:, P : 2 * P], sc[:, P : 2 * P], carL[:, 1:2]
            )
            nc.vector.tensor_scalar_min(
                sc[:, 2 * P : 3 * P], sc[:, 2 * P : 3 * P], carR[:, 0:1]
            )
            nc.vector.tensor_scalar_min(
                sc[:, 3 * P : 4 * P], sc[:, 3 * P : 4 * P], carR[:, 1:2]
            )
            l1m = sb.tile([WP, P], i32)
            nc.vector.tensor_copy(l1m[:], sc[:, 0:P])
            l2m = sb.tile([WP, P], i32)
            nc.vector.tensor_copy(l2m[:], sc[:, P : 2 * P])
            r1m = sb.tile([WP, P], i32)
            nc.vector.tensor_copy(r1m[:], sc[:, 2 * P : 3 * P])
            r2m = sb.tile([WP, P], i32)
            nc.vector.tensor_copy(r2m[:], sc[:, 3 * P : 4 * P])

            slot_l = sb.tile([WP, P], i32)
            nc.vector.tensor_scalar(
                slot_l[:], l1m[:], 7, None, Alu.arith_shift_right
            )
            slot_r = sb.tile([WP, P], i32)
            nc.vector.tensor_scalar(
                slot_r[:], r1m[:], 7, None, Alu.arith_shift_right
            )
            # sd = slot_l + slot_r - 2*pos'  (>0: left closer; <0: right)
            ssum = sb.tile([WP, P], i32)
            nc.vector.tensor_tensor(ssum[:], slot_l[:], slot_r[:], Alu.add)
            sd = sb.tile([WP, P], i32)
            nc.vector.tensor_tensor(sd[:], ssum[:], pos2_i, Alu.subtract)
            # j = 128*c + jp  (u = jp+1 payload -> key stores jp; v = c)
            jcl = sb.tile([WP, P], i32)
            nc.vector.tensor_scalar(
                jcl[:], l2m[:], 63, 7, Alu.bitwise_and, Alu.arith_shift_left
            )
            jpl = sb.tile([WP, P], i32)
            nc.vector.tensor_scalar(jpl[:], l1m[:], 127, None, Alu.bitwise_and)
            jl = sb.tile([WP, P], i32)
            nc.vector.tensor_tensor(jl[:], jcl[:], jpl[:], Alu.bitwise_or)
            jcr = sb.tile([WP, P], i32)
            nc.vector.tensor_scalar(
                jcr[:], r2m[:], 63, 7, Alu.bitwise_and, Alu.arith_shift_left
            )
            jpr = sb.tile([WP, P], i32)
            nc.vector.tensor_scalar(jpr[:], r1m[:], 127, None, Alu.bitwise_and)
            jr = sb.tile([WP, P], i32)
            nc.vector.tensor_tensor(jr[:], jcr[:], jpr[:], Alu.bitwise_or)

            ml = sb.tile([WP, P], i32)
            nc.vector.tensor_scalar(ml[:], sd[:], 0, None, Alu.is_gt)
            mr = sb.tile([WP, P], i32)
            nc.vector.tensor_scalar(mr[:], sd[:], 0, None, Alu.is_lt)
            src = sb.tile([WP, P], i32)
            nc.vector.tensor_tensor(src[:], jl[:], jr[:], Alu.min)
            nc.vector.copy_predicated(src[:], mr[:], jr[:])
            nc.vector.copy_predicated(src[:], ml[:], jl[:])
            srcf = sb.tile([WP, P], f32)
            nc.vector.tensor_copy(srcf[:], src[:])

            # ---- transpose src rows -> [128, WP]; own slice = cols 1..17
            g_ps = ps.tile([P, WP], f32)
            nc.tensor.transpose(g_ps[:], srcf[:], ident)
            g_i = sb.tile([P, 16], i32)
            nc.vector.tensor_copy(g_i[:], g_ps[:, 1 : WP - 1])

            # ---- gather + cast + write, one 128-row block per indirect DMA
            # (multi-column offset APs mis-execute on HW; single-column is
            # proven.  Q7 descriptor generation pipelines with SDMA + writes.)
            yview = y[:].rearrange("(r p) c -> p r c", p=P)
            for k in range(16):
                gt = sb.tile([P, C], bf16, tag=f"gt{k}")
                nc.gpsimd.indirect_dma_start(
                    out=gt[:],
                    out_offset=None,
                    in_=xch[:],
                    in_offset=bass.IndirectOffsetOnAxis(
                        ap=g_i[:, k : k + 1], axis=0
                    ),
                )
                if WRITE_BF16:
                    wsrc = gt[:]
                else:
                    gf = sb.tile([P, C], f32, tag=f"gf{k}")
                    nc.vector.tensor_copy(gf[:], gt[:])
                    wsrc = gf[:]
                weng = nc.sync if k % 2 == 0 else nc.scalar
                weng.dma_start(yview[:, k : k + 1, :], wsrc.unsqueeze(1))

    return {"y": y}


def host_inputs(x_coarse, keep_idx):
    import ml_dtypes

    bf = ml_dtypes.bfloat16
    x32 = np.ascontiguousarray(np.asarray(x_coarse), dtype=np.float32)
    ki = np.ascontiguousarray(np.asarray(keep_idx), dtype=np.int64).reshape(-1)
    assert ki.shape == (E_COARSE,)

    # window-coverage guard: every slot's nearest kept neighbor (each side,
    # where one exists) must lie within the per-core window margin (128).
    kept_sorted = np.sort(ki)
    gaps = np.diff(kept_sorted)
    assert gaps.max() <= P, f"kept gap {gaps.max()} exceeds window margin"
    assert kept_sorted[0] <= P and (E_FINE - 1 - kept_sorted[-1]) <= P

    keep_w = ki.reshape(KC, P).T  # [jp, c] = keep_idx[128c+jp]
    hb = np.full((P, KCP), -1000, dtype=np.int64)
    lb = np.full((P, KCP), -1000, dtype=np.int64)
    hb[:, :KC] = keep_w >> 7
    lb[:, :KC] = keep_w & 127

    t = np.arange(P)
    iota_a = np.tile(t[None, :], (P, 1)).astype(bf)
    crpb_a = np.tile(
        np.minimum(np.arange(KCP, dtype=np.float32), KC - 1)[None, :, None],
        (P, 1, OHW),
    ).astype(bf)
    jpk_a = (t[:, None] + 1).astype(np.float32)
    xch_a = x32.astype(bf)

    base = {
        "xch": xch_a,
        "bpk": np.ascontiguousarray(iota_a),
    }
    if DEVICE_SCATTER:
        base["crpb"] = np.ascontiguousarray(crpb_a)
        base["jpk"] = np.ascontiguousarray(jpk_a)
        base["lbw"] = np.ascontiguousarray(lb.astype(bf))

    in_maps = []
    for m in range(N_CORES):
        wbase = 16 * m - 1
        w = np.arange(WP)
        posp = POS_OFF + P * (wbase + w[:, None]) + t[None, :]
        p128m1 = (128.0 * posp - 1).astype(np.float32)
        p64 = (64.0 * posp).astype(np.float32)
        ident_a = np.eye(WP, dtype=np.float32)
        fpk_a = np.concatenate([p128m1, p64, ident_a], axis=1)
        ipk_a = np.concatenate([posp, 2 * posp], axis=1).astype(np.int32)
        im = dict(
            base,
            fpk=np.ascontiguousarray(fpk_a),
            ipk=np.ascontiguousarray(ipk_a),
        )
        if DEVICE_SCATTER:
            im["hbw"] = np.ascontiguousarray((hb - wbase).astype(bf))
        else:
            # pre-biased scan keys: key1 = 128*pos' + jp, key2 = 64*pos' + c
            tbl_a = np.zeros((WP, 2 * P), dtype=np.float32)
            winm = (hb >= wbase) & (hb < wbase + WP)
            jj, cc = np.nonzero(winm)
            posk = POS_OFF + keep_w[jj, cc]
            tbl_a[hb[jj, cc] - wbase, lb[jj, cc]] = 128 * posk + jj
            tbl_a[hb[jj, cc] - wbase, P + lb[jj, cc]] = 64 * posk + cc
            im["tbl"] = np.ascontiguousarray(tbl_a)
        in_maps.append(im)
    return in_maps


def assemble_output(res):
    out = np.concatenate(
        [np.asarray(res.results[m]["y"]) for m in range(N_CORES)], axis=0
    )
    return np.ascontiguousarray(out.astype(np.float32))


def _get_nc():
    if "nc" in _NC_CACHE:
        return _NC_CACHE["nc"]
    _ensure_paths()
    from concourse import bass, mybir
    import concourse.bacc as bacc
    import concourse.tile as tile

    nc = bacc.Bacc(
        "TRN2", target_bir_lowering=False, debug=False, dynamic_dma_scratch_size=16384
    )
    build_program(nc, bass, mybir, tile)
    nc.compile()
    _NC_CACHE["nc"] = nc
    return nc


def run_on_hw(in_maps, trace=False, **kwargs):
    _ensure_paths()
    from concourse.bass_utils import run_bass_kernel_spmd

    nc = _get_nc()
    return run_bass_kernel_spmd(
        nc, in_maps, core_ids=list(range(N_CORES)), trace=trace, **kwargs
    )


def kernel(x_coarse, keep_idx, E_fine=None, **_unused):
    in_maps = host_inputs(x_coarse, keep_idx)
    res = run_on_hw(in_maps)
    return assemble_output(res)
